# revision 54
# baseline (speedup 1.0000x reference)
"""Trainium2 Bass kernel for causal attention with RoPE (GPT-style block).

Shapes (hardcoded): x [2, 2048, 1024], w_attn [1024, 3072], b_attn [3072],
w_proj [1024, 1024], b_proj [1024]. 16 heads, head dim 64, rope theta 1e4.

Sharding over 8 cores: core c = 4*b + hg handles batch b (2-way data
parallel) and head group hg (4 heads, tensor parallel). After attention, an
8-way AllToAll (split into two head-pair halves so the first overlaps
pair-1 attention compute) redistributes y^T in T-eighths: core c ends up
with t-columns [256c, 256c+256) of BOTH batches and runs the output
projection for them — no all-reduce needed anywhere.

All matmuls run as float32r (full PE rate at free-dim >= 256, ~6e-5 rounding)
with fp32 PSUM accumulation; softmax skips max-subtraction (logits are
bounded ~±3 for this init scale) and gets its denominators from a ones
column appended to V, so no P^T transposes and no partition reductions.
"""

import sys

sys.path.insert(0, "/opt/trn_rl_repo")

from contextlib import ExitStack

import numpy as np

import concourse.mybir as mybir
import concourse.tile as tile
from concourse import bacc
from concourse.bass_utils import run_bass_kernel_spmd

F32 = mybir.dt.float32
F32R = mybir.dt.float32r
AF = mybir.ActivationFunctionType

N_CORES = 8
B, T, C = 2, 2048, 1024
H, D = 16, 64
HPC = 4  # heads per core
ROPE_THETA = 10000.0
P = 128
NQ = 4  # 512-wide q slices
QW = 512  # q slice width
NKT = 16  # 128-row k tiles
NCT = 8  # C/128 contraction tiles

_CACHED = {}


def _build_nc(sim_mode=False):
    """sim_mode=True replaces the collectives with local DMAs so the
    single-core TimelineSim (cost model) can run."""
    nc = bacc.Bacc("TRN2", target_bir_lowering=False, debug=False)

    xT = nc.dram_tensor("xT", [C, T], F32, kind="ExternalInput")
    w_qk = nc.dram_tensor("w_qk", [C, 512], F32, kind="ExternalInput")
    w_v = nc.dram_tensor("w_v", [C, 256], F32, kind="ExternalInput")
    w_pr = nc.dram_tensor("w_pr", [C, C], F32, kind="ExternalInput")
    b_qk = nc.dram_tensor("b_qk", [512, 1], F32, kind="ExternalInput")
    b_pr = nc.dram_tensor("b_pr", [1, C], F32, kind="ExternalInput")
    cos2 = nc.dram_tensor("cos2", [P, T], F32, kind="ExternalInput")
    sin2 = nc.dram_tensor("sin2", [P, T], F32, kind="ExternalInput")
    p128t = nc.dram_tensor("p128t", [P, P], F32, kind="ExternalInput")
    tri = nc.dram_tensor("tri", [P, P], F32, kind="ExternalInput")
    outT = nc.dram_tensor("outT", [C, QW], F32, kind="ExternalOutput")

    # all-to-all over all 8 cores: chunk d = my 4 heads' y^T for t-eighth d,
    # split by head pair so pair 0's exchange overlaps pair-1 attention.
    snd = [nc.dram_tensor(f"snd{h}", [8, P, 256], F32) for h in range(2)]
    rcv = [nc.dram_tensor(f"rcv{h}", [8, P, 256], F32) for h in range(2)]

    with tile.TileContext(nc) as tc, ExitStack() as octx:
        cpool = octx.enter_context(tc.tile_pool(name="cpool", bufs=1))
        cos_sb = cpool.tile([P, T], F32, tag="cos")
        sin_sb = cpool.tile([P, T], F32, tag="sin")
        pswap_sb = cpool.tile([P, P], F32R, tag="pswap")
        tri_sb = cpool.tile([P, P], F32R, tag="tri")
        bqk_sb = cpool.tile([P, 4], F32, tag="bqk")
        bpr_sb = cpool.tile([P, 8], F32, tag="bpr")
        qk_sb = [cpool.tile([P, T], F32R, tag=f"qk{m}", name=f"qk{m}") for m in range(4)]
        vaug_sb = [
            cpool.tile([P, 4 * 65], F32R, tag=f"va{t}", name=f"va{t}")
            for t in range(NKT)
        ]
        wpr_big = cpool.tile([P, NCT * C], F32R, tag="wprbig")

        # ---------------- phase 1: qkv projection + rope -----------------
        with ExitStack() as ctx:
            xw = ctx.enter_context(tc.tile_pool(name="xw", bufs=1))
            wk1 = ctx.enter_context(tc.tile_pool(name="wk1", bufs=4))
            ps1 = ctx.enter_context(tc.tile_pool(name="ps1", bufs=8, space="PSUM"))

            xt_sb = [xw.tile([P, T], F32R, tag=f"xt{k}", name=f"xt{k}") for k in range(NCT)]
            wqk_sb = [xw.tile([P, 512], F32R, tag=f"wqk{k}", name=f"wqk{k}") for k in range(NCT)]
            wv_big = xw.tile([P, NCT * 256], F32R, tag="wvbig")

            # critical stream: (wqk[k], xt[k]) pairs, then cos/sin for rope
            for k in range(NCT):
                nc.sync.dma_start(
                    wqk_sb[k][:], w_qk[P * k : P * (k + 1), :].bitcast(F32R)
                )
                nc.sync.dma_start(xt_sb[k][:], xT[P * k : P * (k + 1), :].bitcast(F32R))
            # b_qk [512,1] -> [128, 4] (col m = rows 128m)
            nc.sync.dma_start(
                bqk_sb[:].rearrange("p (m o) -> p m o", m=4),
                b_qk[:].rearrange("(m p) o -> p m o", m=4),
            )
            # w_v [1024, 256] -> [128, 8*256]
            nc.sync.dma_start(
                wv_big[:].rearrange("p (k c) -> p k c", k=NCT),
                w_v[:].rearrange("(k p) c -> p k c", k=NCT).bitcast(F32R),
            )
            nc.sync.dma_start(pswap_sb[:], p128t[:].bitcast(F32R))
            nc.sync.dma_start(tri_sb[:], tri[:].bitcast(F32R))
            nc.sync.dma_start(cos_sb[:], cos2[:])
            nc.sync.dma_start(sin_sb[:], sin2[:])
            nc.sync.dma_start(
                bpr_sb[:].unsqueeze(2),
                b_pr[:].rearrange("o (m p) -> p m o", m=8),
            )

            # qkT [512 rows, T] = w_qk.T @ x.T; k-outer waves of TWO n-slices
            # (8 accumulators = all 8 psum banks) so wave A, paced by xt[k]
            # DMA arrival, keeps PE ~50% fed. All phase-1 psum tiles share
            # one tag (one [128,512] bank slot each, 8 bufs).
            v_accs = {}

            def qk_wave(ns):
                accs = {
                    (n, m): ps1.tile([P, QW], F32, tag="bank", name=f"qkps{n}_{m}")
                    for n in ns
                    for m in range(4)
                }
                for k in range(NCT):
                    for n in ns:
                        for m in range(4):
                            nc.tensor.matmul(
                                accs[(n, m)][:],
                                wqk_sb[k][:, P * m : P * (m + 1)],
                                xt_sb[k][:, QW * n : QW * (n + 1)],
                                start=(k == 0),
                                stop=(k == NCT - 1),
                            )
                for n in ns:
                    for m in range(4):
                        nc.scalar.activation(
                            qk_sb[m][:, QW * n : QW * (n + 1)],
                            accs[(n, m)][:],
                            AF.Identity,
                            bias=bqk_sb[:, m : m + 1],
                        )

            def v_store(t):
                acc = v_accs.pop(t)
                va = vaug_sb[t][:].rearrange("p (h d) -> p h d", h=4)
                nc.scalar.activation(
                    va[:, :, 0:64],
                    acc.rearrange("p (h d) -> p h d", h=4),
                    AF.Copy,
                )
                # ones column: Identity(0*in + 1). Read a KNOWN tile (bqk),
                # not the tile itself: 0 * garbage-NaN would poison the ones.
                nc.scalar.activation(
                    va[:, :, 64:65],
                    bqk_sb[:, 0:1].unsqueeze(1).broadcast_to((P, 4, 1)),
                    AF.Identity,
                    bias=1.0,
                    scale=0.0,
                )

            def v_wave(ts):
                vacc = {
                    t: ps1.tile([P, 256], F32, tag="bank", name=f"vps{t}")[:]
                    for t in ts
                }
                for k in range(NCT):
                    for t in ts:
                        nc.tensor.matmul(
                            vacc[t],
                            xt_sb[k][:, P * t : P * (t + 1)],
                            wv_big[:, 256 * k : 256 * (k + 1)],
                            start=(k == 0),
                            stop=(k == NCT - 1),
                        )
                for t in ts:
                    v_accs[t] = vacc[t]
                for t in ts:
                    v_store(t)

            rot_sb = {}

            def rot_batch(ms, nrange=None):
                for m in ms:
                    for n in nrange if nrange is not None else range(NQ):
                        sl = slice(QW * n, QW * (n + 1))
                        rot = ps1.tile([P, QW], F32, tag="bank", name=f"rot{m}_{n}")
                        nc.tensor.matmul(
                            rot[:], pswap_sb[:], qk_sb[m][:, sl], start=True, stop=True
                        )
                        rs = wk1.tile([P, QW], F32, tag="rotsb", name=f"rots{m}_{n}")
                        nc.scalar.activation(rs[:], rot[:], AF.Copy)
                        rot_sb[(m, n)] = rs

            qk_wave((0, 1))
            qk_wave((2, 3))
            rot_batch((0, 2))  # q/k heads 0-1 first: attention pair 0 unblocks
            v_wave(range(0, 4))
            rot_batch((1, 3))
            v_wave(range(4, 8))
            v_wave(range(8, 12))
            v_wave(range(12, 16))

            # rope combine, split across DVE (sin mul + add) and Pool (cos mul)
            for m in (0, 2, 1, 3):
                for n in range(NQ):
                    sl = slice(QW * n, QW * (n + 1))
                    tmp = wk1.tile([P, QW], F32, tag="ropetmp", name=f"rt{m}_{n}")
                    nc.vector.tensor_tensor(
                        tmp[:], rot_sb[(m, n)][:], sin_sb[:, sl], mybir.AluOpType.mult
                    )
                    nc.gpsimd.tensor_tensor(
                        qk_sb[m][:, sl],
                        qk_sb[m][:, sl],
                        cos_sb[:, sl],
                        mybir.AluOpType.mult,
                    )
                    nc.vector.tensor_tensor(
                        qk_sb[m][:, sl], qk_sb[m][:, sl], tmp[:], mybir.AluOpType.add
                    )

        # ---------------- phase 2: attention -----------------
        with ExitStack() as ctx:
            pp = ctx.enter_context(tc.tile_pool(name="pp", bufs=6))
            wk2 = ctx.enter_context(tc.tile_pool(name="wk2", bufs=3))
            pss = ctx.enter_context(tc.tile_pool(name="pss", bufs=3, space="PSUM"))
            psy = ctx.enter_context(tc.tile_pool(name="psy", bufs=2, space="PSUM"))

            # prefetch w_proj now: DMA queues are mostly idle during attention
            nc.sync.dma_start(
                wpr_big[:].rearrange("p (k c) -> p k c", k=NCT),
                w_pr[:].rearrange("(k p) c -> p k c", k=NCT).bitcast(F32R),
            )

            for pair in range(2):
                qtile = qk_sb[pair]
                ktile = qk_sb[2 + pair]
                for i in range(NQ):
                    ypsAB = [
                        psy.tile([65, QW], F32, tag="yps", name=f"yps{pair}_{i}_{_h}")
                        for _h in range(2)
                    ]
                    jmax = 4 * i + 3

                    def s_block(j):
                        # both heads' S^T blocks in one [128, 1024] psum tile,
                        # then exp (+ causal strip mask on diagonal blocks)
                        o = max(0, j - 4 * i)
                        w0 = P * o
                        qsl = slice(QW * i + w0, QW * (i + 1))
                        ksl = slice(P * j, P * (j + 1))
                        sps = pss.tile(
                            [P, 2 * QW], F32, tag="sps", name=f"sps{pair}_{i}_{j}"
                        )
                        for hh in range(2):
                            rows = slice(64 * hh, 64 * (hh + 1))
                            nc.tensor.matmul(
                                sps[:, QW * hh + w0 : QW * (hh + 1)],
                                ktile[rows, ksl],
                                qtile[rows, qsl],
                                start=True,
                                stop=True,
                            )
                        pt = pp.tile(
                            [P, 2 * QW], F32R, tag="pt", name=f"pt{pair}_{i}_{j}"
                        )
                        spsv = sps[:].rearrange("p (h q) -> p h q", h=2)
                        ptv = pt[:].rearrange("p (h q) -> p h q", h=2)
                        nc.scalar.activation(
                            ptv[:, :, w0:QW], spsv[:, :, w0:QW], AF.Exp, scale=0.125
                        )
                        if j >= 4 * i:
                            stv = ptv[:, :, w0 : w0 + P]
                            nc.vector.tensor_tensor(
                                stv,
                                stv,
                                tri_sb[:].unsqueeze(1).broadcast_to((P, 2, P)),
                                mybir.AluOpType.mult,
                            )
                        return pt

                    def pv_block(j, pt):
                        o = max(0, j - 4 * i)
                        w0 = P * o
                        win = slice(w0, QW)
                        for hh in range(2):
                            a = 2 * pair + hh
                            nc.tensor.matmul(
                                ypsAB[hh][:, win],
                                vaug_sb[j][:, 65 * a : 65 * a + 65],
                                pt[:, QW * hh + w0 : QW * (hh + 1)],
                                start=(j == 0),
                                stop=(j == jmax),
                            )

                    # software pipeline: S(j+1) issues on PE before PV(j) so
                    # PE never stalls waiting for exp(j) on ACT
                    pts = {0: s_block(0)}
                    for j in range(jmax + 1):
                        if j + 1 <= jmax:
                            pts[j + 1] = s_block(j + 1)
                        pv_block(j, pts.pop(j))

                    for hh in range(2):
                        yps = ypsAB[hh]
                        # copy psum -> sbuf first: frees the psum bank after
                        # ~0.7us instead of after the whole normalize chain
                        yc = wk2.tile([65, QW], F32, tag="yc", name=f"yc{pair}_{i}_{hh}")
                        nc.vector.tensor_copy(yc[:], yps[:])
                        r_sb = wk2.tile([1, QW], F32, tag="rsb", name=f"r{pair}_{i}_{hh}")
                        nc.vector.reciprocal(r_sb[:], yc[64:65, :])
                        rbc = wk2.tile([64, QW], F32, tag="rbc", name=f"rb{pair}_{i}_{hh}")
                        nc.gpsimd.partition_broadcast(rbc[:], r_sb[:])
                        yt_sb = wk2.tile([64, QW], F32, tag="ytsb", name=f"yt{pair}_{i}_{hh}")
                        nc.vector.tensor_tensor(
                            yt_sb[:], yc[0:64, :], rbc[:], mybir.AluOpType.mult
                        )
                        # one DMA into both t-eighth chunks (2i, 2i+1):
                        # dst iterates [p, d, c] (partition dim first on SBUF)
                        dst = (
                            snd[pair][2 * i : 2 * i + 2, 64 * hh : 64 * hh + 64, :]
                            .rearrange("d p c -> p d c")
                        )
                        nc.sync.dma_start(dst, yt_sb[:].rearrange("p (d c) -> p d c", d=2))

                # all-to-all for this head pair; pair 0's overlaps pair 1
                if sim_mode:
                    nc.sync.dma_start(rcv[pair][:], snd[pair][:])
                else:
                    nc.gpsimd.collective_compute(
                        "AllToAll",
                        mybir.AluOpType.bypass,
                        replica_groups=[[0, 1, 2, 3, 4, 5, 6, 7]],
                        ins=[snd[pair][:]],
                        outs=[rcv[pair][:]],
                    )

        # ---------------- phase 4: output projection -----------------
        # y^T rows 128k: source s = k//2, head-pair half hp = k%2. Pack the
        # four k-tiles of each half into one [128, 2048] tile (cols 512*j for
        # k = 2j + hp; first 256 batch 0, next 256 batch 1), loaded by a
        # single DMA per half. k-outer accumulation: all pair-0 contributions
        # run before the second all-to-all lands.
        with ExitStack() as ctx:
            pj = ctx.enter_context(tc.tile_pool(name="pj", bufs=1))
            wk3 = ctx.enter_context(tc.tile_pool(name="wk3", bufs=1))
            pso = ctx.enter_context(tc.tile_pool(name="pso", bufs=1, space="PSUM"))

            yhalf = [pj.tile([P, 4 * QW], F32R, tag=f"yh{hp}", name=f"yh{hp}") for hp in range(2)]
            for hp in range(2):
                for b in range(2):
                    # dst [p, j, c] (col 512j + 256b + c) <- rcv[hp][4b+j, p, c]
                    dstv = yhalf[hp][:].rearrange("p (j c) -> p j c", j=4)[
                        :, :, 256 * b : 256 * b + 256
                    ]
                    srcv = rcv[hp][4 * b : 4 * b + 4, :, :].rearrange(
                        "j p c -> p j c"
                    ).bitcast(F32R)
                    nc.sync.dma_start(dstv, srcv)
            accs = [
                pso.tile([P, QW], F32, tag=f"ops{m}", name=f"ops{m}") for m in range(8)
            ]
            for ki, (hp, j) in enumerate(
                [(0, 0), (0, 1), (0, 2), (0, 3), (1, 0), (1, 1), (1, 2), (1, 3)]
            ):
                k = 2 * j + hp
                for m in range(8):
                    nc.tensor.matmul(
                        accs[m][:],
                        wpr_big[:, C * k + P * m : C * k + P * (m + 1)],
                        yhalf[hp][:, QW * j : QW * (j + 1)],
                        start=(ki == 0),
                        stop=(ki == 7),
                    )
            obig = wk3.tile([P, 8 * QW], F32, tag="obig")
            for m in range(8):
                dst = obig[:, QW * m : QW * (m + 1)]
                if m % 2 == 0:
                    nc.scalar.activation(
                        dst, accs[m][:], AF.Identity, bias=bpr_sb[:, m : m + 1]
                    )
                else:
                    nc.vector.tensor_scalar_add(dst, accs[m][:], bpr_sb[:, m : m + 1])
            for half in range(2):
                nc.sync.dma_start(
                    outT[4 * P * half : 4 * P * (half + 1), :].rearrange(
                        "(m p) c -> p m c", m=4
                    ),
                    obig[:, 4 * QW * half : 4 * QW * (half + 1)].rearrange(
                        "p (m c) -> p m c", m=4
                    ),
                )

    nc.compile()
    return nc


def _host_prep(x, w_attn, b_attn, w_proj, b_proj):
    """Build the 8 per-core input dicts."""
    # rope tables in [D, T] layout, 2 heads stacked (identical pattern)
    inv = ROPE_THETA ** (-np.arange(0, D, 2, dtype=np.float64) / D)  # [32]
    f = np.arange(T, dtype=np.float64)[:, None] * inv[None, :]  # [T, 32]
    cos_dT = np.repeat(np.cos(f).T, 2, axis=0).astype(np.float32)  # [64, T]
    sin_dT = np.repeat(np.sin(f).T, 2, axis=0).astype(np.float32)
    cos2 = np.ascontiguousarray(np.tile(cos_dT, (2, 1)))  # [128, T]
    sin2 = np.ascontiguousarray(np.tile(sin_dT, (2, 1)))

    p64 = np.zeros((64, 64), np.float32)
    for q in range(32):
        p64[2 * q, 2 * q + 1] = -1.0
        p64[2 * q + 1, 2 * q] = 1.0
    p128 = np.zeros((128, 128), np.float32)
    p128[:64, :64] = p64
    p128[64:, 64:] = p64
    p128t = np.ascontiguousarray(p128.T)

    kk = np.arange(P)[:, None]
    ss = np.arange(P)[None, :]
    tri = (ss >= kk).astype(np.float32)

    in_maps = []
    for c in range(N_CORES):
        b, hg = divmod(c, 4)
        qc = slice(256 * hg, 256 * hg + 256)
        kc = slice(C + 256 * hg, C + 256 * hg + 256)
        vc = slice(2 * C + 256 * hg, 2 * C + 256 * hg + 256)
        in_maps.append(
            {
                "xT": np.ascontiguousarray(x[b].T),
                "w_qk": np.ascontiguousarray(
                    np.concatenate([w_attn[:, qc], w_attn[:, kc]], axis=1)
                ),
                "w_v": np.ascontiguousarray(w_attn[:, vc]),
                "w_pr": np.ascontiguousarray(w_proj),
                "b_qk": np.ascontiguousarray(
                    np.concatenate([b_attn[qc], b_attn[kc]]).reshape(512, 1)
                ),
                "b_pr": np.ascontiguousarray(b_proj.reshape(1, C)),
                "cos2": cos2,
                "sin2": sin2,
                "p128t": p128t,
                "tri": tri,
            }
        )
    return in_maps


def run(x, w_attn, b_attn, w_proj, b_proj, trace=False, trace_cores=None):
    x = np.asarray(x, np.float32)
    w_attn = np.asarray(w_attn, np.float32)
    b_attn = np.asarray(b_attn, np.float32)
    w_proj = np.asarray(w_proj, np.float32)
    b_proj = np.asarray(b_proj, np.float32)

    if "nc" not in _CACHED:
        _CACHED["nc"] = _build_nc()
    nc = _CACHED["nc"]

    in_maps = _host_prep(x, w_attn, b_attn, w_proj, b_proj)
    res = run_bass_kernel_spmd(
        nc,
        in_maps,
        list(range(N_CORES)),
        trace=trace,
        trace_cores=trace_cores,
    )

    # v-bias contributes exactly  b_v @ w_proj  to every output row
    bvproj = (b_attn[2 * C :].astype(np.float64) @ w_proj.astype(np.float64)).astype(
        np.float32
    )
    out = np.empty((B, T, C), np.float32)
    for c in range(N_CORES):
        oT = res.results[c]["outT"]  # [1024, 512]: cols 0:256 batch0, 256:512 batch1
        out[0, 256 * c : 256 * (c + 1), :] = oT[:, 0:256].T
        out[1, 256 * c : 256 * (c + 1), :] = oT[:, 256:512].T
    out += bvproj[None, None, :]
    return out, res


def kernel(x, w_attn, b_attn, w_proj, b_proj):
    out, _ = run(x, w_attn, b_attn, w_proj, b_proj, trace=False)
    return out
